# revision 42
# baseline (speedup 1.0000x reference)
"""Multi-head causal attention (B=4, T=2048, D=1024, 16 heads) on 8 TRN2 cores.

Sharding: core c -> batch b = c//2, head-group g = c%2 (8 of 16 heads).
Each core computes its batch's QKV for its heads, flash-style causal
attention with scores kept transposed (S^T[k, q]) so softmax sums come
free via a ones-column appended to V, then a partial output projection
y_part = attn_local @ W_proj[rows]. Host sums the two head-group partials
per batch.

All matmuls run in float32r (full PE rate for N>=256); everything else
is fp32.
"""

import math
from contextlib import ExitStack

import numpy as np

import concourse.bacc as bacc
import concourse.bass as bass
import concourse.mybir as mybir
import concourse.tile as tile
from concourse.bass_utils import run_bass_kernel_spmd

AF = mybir.ActivationFunctionType
F32 = mybir.dt.float32
F32R = mybir.dt.float32r
F16 = mybir.dt.float16
U16 = mybir.dt.uint16

B_FULL = 4
T_FULL = 2048
D_FULL = 1024
NH_FULL = 16
HD = 64


def build_program(T, D, HL, n_pat, blocks):
    """Build the per-core SPMD program.

    T: sequence length, D: model dim, HL: local heads, n_pat: number of
    distinct mixed-mask pattern tiles, blocks: per q-chunk list of
    (k_tile_index, pattern_index_or_None) for active score blocks.
    """
    CL = HL * HD            # local channels (q, k, or v width)
    NDT = D // 128          # d-tiles (contraction tiles for qkv matmuls)
    NTT = T // 128          # t-tiles
    QCW = min(512, T)       # q-chunk width
    NQC = T // QCW
    TPC = QCW // 128        # t-tiles per q-chunk
    NCT = CL // 128         # c-tiles for q/k/attn storage
    PCH = min(512, D)       # proj output chunk
    NPCH = D // PCH
    scale = 1.0 / math.sqrt(HD)

    nc = bacc.Bacc("TRN2", target_bir_lowering=False, debug=False)
    x = nc.dram_tensor("x", [T, D], F16, kind="ExternalInput").ap()
    wq = nc.dram_tensor("wq", [D, CL], F16, kind="ExternalInput").ap()
    wk = nc.dram_tensor("wk", [D, CL], F16, kind="ExternalInput").ap()
    wv = nc.dram_tensor("wv", [D, CL], F16, kind="ExternalInput").ap()
    bq = nc.dram_tensor("bq", [CL], F32, kind="ExternalInput").ap()
    bk = nc.dram_tensor("bk", [CL], F32, kind="ExternalInput").ap()
    bv = nc.dram_tensor("bv", [CL], F32, kind="ExternalInput").ap()
    wp = nc.dram_tensor("wp", [CL, D], F16, kind="ExternalInput").ap()
    bp = nc.dram_tensor("bp", [D], F32, kind="ExternalInput").ap()
    mp = nc.dram_tensor("mp", [max(n_pat, 1), 128, QCW], F16, kind="ExternalInput").ap()
    y = nc.dram_tensor("y", [T, D], F32, kind="ExternalOutput").ap()

    with tile.TileContext(nc) as tc, nc.allow_low_precision(
        reason="float32r tiles hold full-fp32 data; matmul rounds internally"
    ):
        with ExitStack() as octx:
            persist = octx.enter_context(tc.tile_pool(name="persist", bufs=1))
            kT = [persist.tile([128, T], F16, name=f"kT{i}", tag=f"kT{i}") for i in range(NCT)]
            # Q^T stored twice, zero-padded per head parity, so the scores
            # matmul can contract over the full 128 partitions (the f32r fast
            # path needs K > 64; zero rows kill the other head's channels).
            qTe = [persist.tile([128, T], F16, name=f"qTe{i}", tag=f"qTe{i}") for i in range(NCT)]
            qTo = [persist.tile([128, T], F16, name=f"qTo{i}", tag=f"qTo{i}") for i in range(NCT)]
            for i in range(NCT):
                nc.gpsimd.memset(qTe[i][HD:128, :], 0.0)
                nc.gpsimd.memset(qTo[i][0:HD, :], 0.0)
            # per-head stride 128 elements (256B) keeps the AV stationary
            # loads FWL-aligned; cols [65:128) of each head slot are junk.
            VSW = HL * 128
            vS = [persist.tile([128, VSW], F16, name=f"vS{i}", tag=f"vS{i}") for i in range(NTT)]
            for i in range(NTT):
                nc.gpsimd.memset(vS[i], 0.0)
                nc.gpsimd.memset(
                    vS[i].rearrange("p (h c) -> p h c", c=128)[:, :, HD:HD + 1], 1.0
                )
            attnT = [persist.tile([128, T], F16, name=f"attnT{i}", tag=f"attnT{i}") for i in range(NCT)]
            bqs = persist.tile([128, NCT], F32, name="bqs", tag="bqs")
            bks = persist.tile([128, NCT], F32, name="bks", tag="bks")
            nc.sync.dma_start(out=bqs, in_=bq.rearrange("(m p) -> p m", p=128))
            nc.sync.dma_start(out=bks, in_=bk.rearrange("(m p) -> p m", p=128))
            bvb = persist.tile([128, CL], F32, name="bvb", tag="bvb")
            nc.sync.dma_start(
                out=bvb,
                in_=bass.AP(tensor=bv.tensor, offset=bv.offset, ap=[[0, 128]] + list(bv.ap)),
            )

            # ---- Phase ABC: x^T (per chunk), V natural, Q^T/K^T ----
            with ExitStack() as actx:
                abc = actx.enter_context(tc.tile_pool(name="abc", bufs=2))
                wtp = actx.enter_context(tc.tile_pool(name="wtp", bufs=3))
                wvp = actx.enter_context(tc.tile_pool(name="wvp", bufs=1))
                psb = actx.enter_context(tc.tile_pool(name="psb", bufs=2, space="PSUM"))

                wvt = [wvp.tile([128, CL], F16, name=f"wvt{k}", tag=f"wvt{k}") for k in range(NDT)]
                for kd in range(NDT):
                    nc.sync.dma_start(out=wvt[kd], in_=wv[kd * 128:(kd + 1) * 128, :])

                for ntc in range(NQC):
                    # x^T chunk via xbar DMA transpose (x arrives as fp16)
                    xTc = abc.tile([128, NDT, QCW], F16, name="xTc", tag="xTc")
                    for dd in range(NDT):
                        nc.sync.dma_start_transpose(
                            xTc[:, dd, :],
                            x[ntc * QCW:(ntc + 1) * QCW, dd * 128:(dd + 1) * 128],
                        )
                    for tv in range(TPC):
                        tt = ntc * TPC + tv
                        pv = psb.tile([128, CL], F32, name="pv", tag="pv")
                        for dd in range(NDT):
                            nc.tensor.matmul(
                                pv,
                                lhsT=xTc[:, dd, tv * 128:(tv + 1) * 128],
                                rhs=wvt[dd],
                                start=(dd == 0),
                                stop=(dd == NDT - 1),
                            )
                        nc.vector.tensor_add(
                            vS[tt].rearrange("p (h c) -> p h c", c=128)[:, :, 0:HD],
                            pv.rearrange("p (h d) -> p h d", h=HL),
                            bvb.rearrange("p (h d) -> p h d", h=HL),
                        )
                    for mi in range(2 * NCT):
                        isq = mi < NCT
                        mc = mi % NCT
                        wsrc = wq if isq else wk
                        wt = wtp.tile([128, NDT, 128], F16, name="wt", tag="wt")
                        nc.sync.dma_start(
                            out=wt,
                            in_=wsrc[:, mc * 128:(mc + 1) * 128].rearrange("(n p) c -> p n c", p=128),
                        )
                        pb = psb.tile([128, QCW], F32, name="pb", tag="pb")
                        for dd in range(NDT):
                            nc.tensor.matmul(
                                pb,
                                lhsT=wt[:, dd, :],
                                rhs=xTc[:, dd, :],
                                start=(dd == 0),
                                stop=(dd == NDT - 1),
                            )
                        tsl = slice(ntc * QCW, (ntc + 1) * QCW)
                        if isq:
                            nc.vector.tensor_scalar_add(
                                qTe[mc][0:HD, tsl], pb[0:HD, :], bqs[0:HD, mc:mc + 1]
                            )
                            nc.vector.tensor_scalar_add(
                                qTo[mc][HD:128, tsl], pb[HD:128, :], bqs[HD:128, mc:mc + 1]
                            )
                        else:
                            nc.vector.tensor_scalar_add(
                                kT[mc][:, tsl], pb, bks[:, mc:mc + 1]
                            )

            # ---- Phase D+E fused, qc-outer ----
            # For each q-chunk: all heads run flash attention (k-tile pairs ->
            # one [128, 2*QCW] scores psum -> one Exp -> two AV accumulates),
            # then the output projection for that q-chunk's t-tiles runs
            # immediately, interleaving with the next q-chunk's attention.
            # E's psum tiles share the AV pool slots ([128, 512] f32 both).
            with ExitStack() as dctx:
                dp = dctx.enter_context(tc.tile_pool(name="dp", bufs=1))
                ptl = dctx.enter_context(tc.tile_pool(name="ptl", bufs=3))
                recp = dctx.enter_context(tc.tile_pool(name="recp", bufs=2))
                ysb = dctx.enter_context(tc.tile_pool(name="ysb", bufs=3))
                drp = dctx.enter_context(tc.tile_pool(name="drp", bufs=4, space="DRAM"))
                pss = dctx.enter_context(tc.tile_pool(name="pss", bufs=2, space="PSUM"))
                psav = dctx.enter_context(tc.tile_pool(name="psav", bufs=4, space="PSUM"))

                mts = [dp.tile([128, QCW], F16, name=f"mt{i}", tag=f"mt{i}") for i in range(n_pat)]
                for i in range(n_pat):
                    nc.sync.dma_start(out=mts[i], in_=mp[i])
                wps = [dp.tile([128, D], F16, name=f"wps{i}", tag=f"wps{i}") for i in range(NCT)]
                for cc in range(NCT):
                    nc.sync.dma_start(out=wps[cc], in_=wp[cc * 128:(cc + 1) * 128, :])
                bpb = dp.tile([128, D], F32, name="bpb", tag="bpb")
                nc.sync.dma_start(
                    out=bpb,
                    in_=bass.AP(tensor=bp.tensor, offset=bp.offset, ap=[[0, 128]] + list(bp.ap)),
                )

                REC_BATCH = 2  # heads per reciprocal batch (must be < pav bufs)

                for qc in range(NQC):
                    row = blocks[qc]
                    assert row, f"q-chunk {qc} has no active k-tiles"
                    pavs = {}
                    stag = recp.tile([128, QCW], F32, name="stag", tag="stag")
                    nc.gpsimd.memset(stag, 1.0)
                    rinv = recp.tile([128, QCW], F32, name="rinv", tag="rinv")
                    for h in range(HL):
                        mc = h // 2
                        qTp = (qTe if h % 2 == 0 else qTo)[mc]
                        pav = psav.tile([128, QCW], F32, name="pav", tag="pav")
                        pavs[h] = pav
                        for pi in range(0, len(row), 2):
                            pair = row[pi:pi + 2]
                            w = len(pair) * QCW
                            pS = pss.tile([128, 2 * QCW], F32, name="pS", tag="pS")
                            for sj, (ki, _) in enumerate(pair):
                                nc.tensor.matmul(
                                    pS[:, sj * QCW:(sj + 1) * QCW],
                                    lhsT=kT[mc][:, ki * 128:(ki + 1) * 128],
                                    rhs=qTp[:, qc * QCW:(qc + 1) * QCW],
                                    start=True,
                                    stop=True,
                                )
                            pT = ptl.tile([128, 2 * QCW], F16, name="pT", tag="pT")
                            nc.scalar.activation(pT[:, :w], pS[:, :w], AF.Exp, scale=scale)
                            for sj, (ki, pat) in enumerate(pair):
                                sl = pT[:, sj * QCW:(sj + 1) * QCW]
                                if pat is not None:
                                    nc.vector.tensor_mul(sl, sl, mts[pat[1]])
                                nc.tensor.matmul(
                                    pav,
                                    lhsT=vS[ki][:, h * 128:h * 128 + 128],
                                    rhs=sl,
                                    start=(pi == 0 and sj == 0),
                                    stop=(pi + sj == len(row) - 1),
                                )
                        # l row -> stag (32-aligned slot per head in batch)
                        slot = h % REC_BATCH
                        nc.scalar.copy(stag[slot * 32:slot * 32 + 1, :], pav[HD:HD + 1, :])
                        if slot == REC_BATCH - 1 or h == HL - 1:
                            lo_h = h - slot
                            nc.vector.reciprocal(
                                rinv[0:slot * 32 + 1, :], stag[0:slot * 32 + 1, :]
                            )
                            for bh in range(lo_h, h + 1):
                                bslot = bh % REC_BATCH
                                scr = drp.tile([QCW], F32, name="scr", tag="scr")
                                nc.sync.dma_start(out=scr, in_=rinv[bslot * 32:bslot * 32 + 1, :])
                                rbs = recp.tile([HD, QCW], F32, name="rbs", tag="rbs")
                                nc.sync.dma_start(
                                    out=rbs,
                                    in_=bass.AP(tensor=scr.tensor, offset=scr.offset, ap=[[0, HD]] + list(scr.ap)),
                                )
                                nc.vector.tensor_mul(
                                    attnT[bh // 2][(bh % 2) * HD:(bh % 2) * HD + HD, qc * QCW:(qc + 1) * QCW],
                                    pavs[bh][0:HD, :],
                                    rbs,
                                )
                            if h != HL - 1:
                                stag = recp.tile([128, QCW], F32, name="stag", tag="stag")
                                nc.gpsimd.memset(stag, 1.0)
                                rinv = recp.tile([128, QCW], F32, name="rinv", tag="rinv")
                    # ---- projection for this q-chunk's t-tiles ----
                    for tv in range(TPC):
                        tt = qc * TPC + tv
                        yt = ysb.tile([128, D], F32, name="yt", tag="yt")
                        for nch in range(NPCH):
                            py = psav.tile([128, PCH], F32, name="py", tag="pav")
                            for cc in range(NCT):
                                nc.tensor.matmul(
                                    py,
                                    lhsT=attnT[cc][:, tt * 128:(tt + 1) * 128],
                                    rhs=wps[cc][:, nch * PCH:(nch + 1) * PCH],
                                    start=(cc == 0),
                                    stop=(cc == NCT - 1),
                                )
                            nc.vector.tensor_add(
                                yt[:, nch * PCH:(nch + 1) * PCH], py, bpb[:, nch * PCH:(nch + 1) * PCH]
                            )
                        nc.sync.dma_start(out=y[tt * 128:(tt + 1) * 128, :], in_=yt)
    nc.compile()
    return nc


def classify_mask(mask_bool, T):
    """Classify S^T blocks [k-tile 128, q-chunk 512] as skip / full / mixed.

    mask_bool: [T, T] bool, mask_bool[q, k] = attend(q -> k).
    Returns (blocks, patterns): blocks[qc] = list of (ki, pat_idx|None),
    patterns = np.ndarray [n_pat, 128, QCW] float32.
    """
    QCW = min(512, T)
    NQC = T // QCW
    NKT = T // 128
    maskT = mask_bool.T  # [k, q]
    patterns = []
    pat_index = {}
    blocks = []
    for qc in range(NQC):
        row = []
        for ki in range(NKT):
            blk = maskT[ki * 128:(ki + 1) * 128, qc * QCW:(qc + 1) * QCW]
            if not blk.any():
                continue
            if blk.all():
                row.append((ki, None))
                continue
            key = blk.tobytes()
            if key not in pat_index:
                pat_index[key] = len(patterns)
                patterns.append(blk.astype(np.float32))
            row.append((ki, ("pat", pat_index[key])))
        blocks.append(row)
    n_pat = len(patterns)
    if patterns:
        pats = np.stack(patterns)
    else:
        pats = np.zeros((1, 128, QCW), np.float32)
    return blocks, pats, n_pat


_prog_cache = {}


def _get_program(T, D, HL, mask_bool):
    key = (T, D, HL, mask_bool.tobytes())
    if key not in _prog_cache:
        blocks, pats, n_pat = classify_mask(mask_bool, T)
        nc = build_program(T, D, HL, n_pat, blocks)
        _prog_cache[key] = (nc, blocks, pats)
    return _prog_cache[key]


def kernel(x, W_qkv, b_qkv, W_proj, b_proj, mask):
    out, _ = run_attention(x, W_qkv, b_qkv, W_proj, b_proj, mask)
    return out


def run_attention(x, W_qkv, b_qkv, W_proj, b_proj, mask, trace=False):
    x = np.ascontiguousarray(np.asarray(x, dtype=np.float32))
    W_qkv = np.asarray(W_qkv, dtype=np.float32)
    b_qkv = np.asarray(b_qkv, dtype=np.float32)
    W_proj = np.asarray(W_proj, dtype=np.float32)
    b_proj = np.asarray(b_proj, dtype=np.float32)
    Bc, T, D = x.shape
    NH = NH_FULL
    HL = NH // 2  # heads per core (two head-groups)
    CL = HL * HD

    mask_bool = np.asarray(mask)[0, 0] != 0

    nc, blocks, pats = _get_program(T, D, HL, mask_bool)

    in_maps = []
    n_cores = 2 * Bc
    for c in range(n_cores):
        b, g = c // 2, c % 2
        sl = slice(g * CL, (g + 1) * CL)
        in_maps.append({
            "x": np.ascontiguousarray(x[b]).astype(np.float16),
            "wq": np.ascontiguousarray(W_qkv[:, 0 * D:1 * D][:, sl]).astype(np.float16),
            "wk": np.ascontiguousarray(W_qkv[:, 1 * D:2 * D][:, sl]).astype(np.float16),
            "wv": np.ascontiguousarray(W_qkv[:, 2 * D:3 * D][:, sl]).astype(np.float16),
            "bq": np.ascontiguousarray(b_qkv[0 * D:1 * D][sl]),
            "bk": np.ascontiguousarray(b_qkv[1 * D:2 * D][sl]),
            "bv": np.ascontiguousarray(b_qkv[2 * D:3 * D][sl]),
            "wp": np.ascontiguousarray(W_proj[sl, :]).astype(np.float16),
            "bp": b_proj if g == 0 else np.zeros_like(b_proj),
            "mp": pats.astype(np.float16),
        })

    res = run_bass_kernel_spmd(nc, in_maps, list(range(n_cores)), trace=trace)
    out = np.empty((Bc, T, D), np.float32)
    for b in range(Bc):
        out[b] = res.results[2 * b]["y"] + res.results[2 * b + 1]["y"]
    return out, res


# revision 43
# speedup vs baseline: 1.0376x; 1.0376x over previous
"""Multi-head causal attention (B=4, T=2048, D=1024, 16 heads) on 8 TRN2 cores.

Sharding: core c -> batch b = c//2, head-group g = c%2 (8 of 16 heads).
Each core computes its batch's QKV for its heads, flash-style causal
attention with scores kept transposed (S^T[k, q]) so softmax sums come
free via a ones-column appended to V, then a partial output projection
y_part = attn_local @ W_proj[rows]. Host sums the two head-group partials
per batch.

Matmul operands are fp16 (same ~11-bit mantissa as the PE's fast fp32r
mode, but 1-pass FWL weight loads); accumulation stays fp32 in PSUM.
"""

import math
from contextlib import ExitStack

import numpy as np

import concourse.bacc as bacc
import concourse.bass as bass
import concourse.mybir as mybir
import concourse.tile as tile
from concourse.bass_utils import run_bass_kernel_spmd

AF = mybir.ActivationFunctionType
F32 = mybir.dt.float32
F32R = mybir.dt.float32r
F16 = mybir.dt.float16
U16 = mybir.dt.uint16

B_FULL = 4
T_FULL = 2048
D_FULL = 1024
NH_FULL = 16
HD = 64


def build_program(T, D, HL, n_pat, blocks):
    """Build the per-core SPMD program.

    T: sequence length, D: model dim, HL: local heads, n_pat: number of
    distinct mixed-mask pattern tiles, blocks: per q-chunk list of
    (k_tile_index, pattern_index_or_None) for active score blocks.
    """
    CL = HL * HD            # local channels (q, k, or v width)
    NDT = D // 128          # d-tiles (contraction tiles for qkv matmuls)
    NTT = T // 128          # t-tiles
    QCW = min(512, T)       # q-chunk width
    NQC = T // QCW
    TPC = QCW // 128        # t-tiles per q-chunk
    NCT = CL // 128         # c-tiles for q/k/attn storage
    PCH = min(512, D)       # proj output chunk
    NPCH = D // PCH
    scale = 1.0 / math.sqrt(HD)

    nc = bacc.Bacc("TRN2", target_bir_lowering=False, debug=False)
    x = nc.dram_tensor("x", [T, D], F16, kind="ExternalInput").ap()
    wq = nc.dram_tensor("wq", [D, CL], F16, kind="ExternalInput").ap()
    wk = nc.dram_tensor("wk", [D, CL], F16, kind="ExternalInput").ap()
    wv = nc.dram_tensor("wv", [D, CL], F16, kind="ExternalInput").ap()
    bq = nc.dram_tensor("bq", [CL], F32, kind="ExternalInput").ap()
    bk = nc.dram_tensor("bk", [CL], F32, kind="ExternalInput").ap()
    bv = nc.dram_tensor("bv", [CL], F32, kind="ExternalInput").ap()
    wp = nc.dram_tensor("wp", [CL, D], F16, kind="ExternalInput").ap()
    bp = nc.dram_tensor("bp", [D], F32, kind="ExternalInput").ap()
    mp = nc.dram_tensor("mp", [max(n_pat, 1), 128, QCW], F16, kind="ExternalInput").ap()
    y = nc.dram_tensor("y", [T, D], F32, kind="ExternalOutput").ap()

    with tile.TileContext(nc) as tc, nc.allow_low_precision(
        reason="float32r tiles hold full-fp32 data; matmul rounds internally"
    ):
        with ExitStack() as octx:
            persist = octx.enter_context(tc.tile_pool(name="persist", bufs=1))
            kT = [persist.tile([128, T], F16, name=f"kT{i}", tag=f"kT{i}") for i in range(NCT)]
            # Q^T stored twice, zero-padded per head parity, so the scores
            # matmul can contract over the full 128 partitions (the f32r fast
            # path needs K > 64; zero rows kill the other head's channels).
            qTe = [persist.tile([128, T], F16, name=f"qTe{i}", tag=f"qTe{i}") for i in range(NCT)]
            qTo = [persist.tile([128, T], F16, name=f"qTo{i}", tag=f"qTo{i}") for i in range(NCT)]
            for i in range(NCT):
                nc.gpsimd.memset(qTe[i][HD:128, :], 0.0)
                nc.gpsimd.memset(qTo[i][0:HD, :], 0.0)
            # per-head stride 128 elements (256B) keeps the AV stationary
            # loads FWL-aligned; cols [65:128) of each head slot are junk.
            VSW = HL * 128
            vS = [persist.tile([128, VSW], F16, name=f"vS{i}", tag=f"vS{i}") for i in range(NTT)]
            for i in range(NTT):
                nc.gpsimd.memset(vS[i], 0.0)
                nc.gpsimd.memset(
                    vS[i].rearrange("p (h c) -> p h c", c=128)[:, :, HD:HD + 1], 1.0
                )
            attnT = [persist.tile([128, T], F16, name=f"attnT{i}", tag=f"attnT{i}") for i in range(NCT)]
            bqs = persist.tile([128, NCT], F32, name="bqs", tag="bqs")
            bks = persist.tile([128, NCT], F32, name="bks", tag="bks")
            nc.sync.dma_start(out=bqs, in_=bq.rearrange("(m p) -> p m", p=128))
            nc.sync.dma_start(out=bks, in_=bk.rearrange("(m p) -> p m", p=128))
            bvb = persist.tile([128, CL], F32, name="bvb", tag="bvb")
            nc.sync.dma_start(
                out=bvb,
                in_=bass.AP(tensor=bv.tensor, offset=bv.offset, ap=[[0, 128]] + list(bv.ap)),
            )

            # ---- Phase ABC: x^T (per chunk), V natural, Q^T/K^T ----
            with ExitStack() as actx:
                abc = actx.enter_context(tc.tile_pool(name="abc", bufs=2))
                wtp = actx.enter_context(tc.tile_pool(name="wtp", bufs=3))
                wvp = actx.enter_context(tc.tile_pool(name="wvp", bufs=1))
                psb = actx.enter_context(tc.tile_pool(name="psb", bufs=2, space="PSUM"))

                wvt = [wvp.tile([128, CL], F16, name=f"wvt{k}", tag=f"wvt{k}") for k in range(NDT)]
                for kd in range(NDT):
                    nc.sync.dma_start(out=wvt[kd], in_=wv[kd * 128:(kd + 1) * 128, :])

                for ntc in range(NQC):
                    # x^T chunk via xbar DMA transpose (x arrives as fp16)
                    xTc = abc.tile([128, NDT, QCW], F16, name="xTc", tag="xTc")
                    for dd in range(NDT):
                        nc.sync.dma_start_transpose(
                            xTc[:, dd, :],
                            x[ntc * QCW:(ntc + 1) * QCW, dd * 128:(dd + 1) * 128],
                        )
                    for tv in range(TPC):
                        tt = ntc * TPC + tv
                        pv = psb.tile([128, CL], F32, name="pv", tag="pv")
                        for dd in range(NDT):
                            nc.tensor.matmul(
                                pv,
                                lhsT=xTc[:, dd, tv * 128:(tv + 1) * 128],
                                rhs=wvt[dd],
                                start=(dd == 0),
                                stop=(dd == NDT - 1),
                            )
                        nc.vector.tensor_add(
                            vS[tt].rearrange("p (h c) -> p h c", c=128)[:, :, 0:HD],
                            pv.rearrange("p (h d) -> p h d", h=HL),
                            bvb.rearrange("p (h d) -> p h d", h=HL),
                        )
                    for mi in range(2 * NCT):
                        isq = mi < NCT
                        mc = mi % NCT
                        wsrc = wq if isq else wk
                        wt = wtp.tile([128, NDT, 128], F16, name="wt", tag="wt")
                        nc.sync.dma_start(
                            out=wt,
                            in_=wsrc[:, mc * 128:(mc + 1) * 128].rearrange("(n p) c -> p n c", p=128),
                        )
                        pb = psb.tile([128, QCW], F32, name="pb", tag="pb")
                        for dd in range(NDT):
                            nc.tensor.matmul(
                                pb,
                                lhsT=wt[:, dd, :],
                                rhs=xTc[:, dd, :],
                                start=(dd == 0),
                                stop=(dd == NDT - 1),
                            )
                        tsl = slice(ntc * QCW, (ntc + 1) * QCW)
                        if isq:
                            nc.vector.tensor_scalar_add(
                                qTe[mc][0:HD, tsl], pb[0:HD, :], bqs[0:HD, mc:mc + 1]
                            )
                            nc.vector.tensor_scalar_add(
                                qTo[mc][HD:128, tsl], pb[HD:128, :], bqs[HD:128, mc:mc + 1]
                            )
                        else:
                            nc.vector.tensor_scalar_add(
                                kT[mc][:, tsl], pb, bks[:, mc:mc + 1]
                            )

            # ---- Phase D+E fused, qc-outer ----
            # For each q-chunk: all heads run flash attention (k-tile pairs ->
            # one [128, 2*QCW] scores psum -> one Exp -> two AV accumulates),
            # then the output projection for that q-chunk's t-tiles runs
            # immediately, interleaving with the next q-chunk's attention.
            # E's psum tiles share the AV pool slots ([128, 512] f32 both).
            with ExitStack() as dctx:
                dp = dctx.enter_context(tc.tile_pool(name="dp", bufs=1))
                ptl = dctx.enter_context(tc.tile_pool(name="ptl", bufs=3))
                recp = dctx.enter_context(tc.tile_pool(name="recp", bufs=2))
                ysb = dctx.enter_context(tc.tile_pool(name="ysb", bufs=3))
                drp = dctx.enter_context(tc.tile_pool(name="drp", bufs=4, space="DRAM"))
                pss = dctx.enter_context(tc.tile_pool(name="pss", bufs=2, space="PSUM"))
                psav = dctx.enter_context(tc.tile_pool(name="psav", bufs=4, space="PSUM"))

                mts = [dp.tile([128, QCW], F16, name=f"mt{i}", tag=f"mt{i}") for i in range(n_pat)]
                for i in range(n_pat):
                    nc.sync.dma_start(out=mts[i], in_=mp[i])
                wps = [dp.tile([128, D], F16, name=f"wps{i}", tag=f"wps{i}") for i in range(NCT)]
                for cc in range(NCT):
                    nc.sync.dma_start(out=wps[cc], in_=wp[cc * 128:(cc + 1) * 128, :])
                bpb = dp.tile([128, D], F32, name="bpb", tag="bpb")
                nc.sync.dma_start(
                    out=bpb,
                    in_=bass.AP(tensor=bp.tensor, offset=bp.offset, ap=[[0, 128]] + list(bp.ap)),
                )

                REC_BATCH = 2  # heads per reciprocal batch (must be < pav bufs)

                for qc in range(NQC):
                    row = blocks[qc]
                    assert row, f"q-chunk {qc} has no active k-tiles"
                    pavs = {}
                    stag = recp.tile([128, QCW], F32, name="stag", tag="stag")
                    nc.gpsimd.memset(stag, 1.0)
                    rinv = recp.tile([128, QCW], F32, name="rinv", tag="rinv")
                    for h in range(HL):
                        mc = h // 2
                        qTp = (qTe if h % 2 == 0 else qTo)[mc]
                        pav = psav.tile([128, QCW], F32, name="pav", tag="pav")
                        pavs[h] = pav
                        for pi in range(0, len(row), 2):
                            pair = row[pi:pi + 2]
                            w = len(pair) * QCW
                            pS = pss.tile([128, 2 * QCW], F32, name="pS", tag="pS")
                            for sj, (ki, _) in enumerate(pair):
                                nc.tensor.matmul(
                                    pS[:, sj * QCW:(sj + 1) * QCW],
                                    lhsT=kT[mc][:, ki * 128:(ki + 1) * 128],
                                    rhs=qTp[:, qc * QCW:(qc + 1) * QCW],
                                    start=True,
                                    stop=True,
                                )
                            pT = ptl.tile([128, 2 * QCW], F16, name="pT", tag="pT")
                            nc.scalar.activation(pT[:, :w], pS[:, :w], AF.Exp, scale=scale)
                            for sj, (ki, pat) in enumerate(pair):
                                sl = pT[:, sj * QCW:(sj + 1) * QCW]
                                if pat is not None:
                                    kind, arg = pat
                                    if kind == "tri":
                                        # keep where (q - k) >= 0, else 0
                                        nc.gpsimd.affine_select(
                                            out=sl,
                                            in_=sl,
                                            pattern=[[1, QCW]],
                                            base=arg,
                                            channel_multiplier=-1,
                                            compare_op=mybir.AluOpType.is_ge,
                                            fill=0.0,
                                        )
                                    else:
                                        nc.gpsimd.tensor_mul(sl, sl, mts[arg])
                                nc.tensor.matmul(
                                    pav,
                                    lhsT=vS[ki][:, h * 128:h * 128 + 128],
                                    rhs=sl,
                                    start=(pi == 0 and sj == 0),
                                    stop=(pi + sj == len(row) - 1),
                                )
                        # l row -> stag (32-aligned slot per head in batch)
                        slot = h % REC_BATCH
                        nc.scalar.copy(stag[slot * 32:slot * 32 + 1, :], pav[HD:HD + 1, :])
                        if slot == REC_BATCH - 1 or h == HL - 1:
                            lo_h = h - slot
                            nc.vector.reciprocal(
                                rinv[0:slot * 32 + 1, :], stag[0:slot * 32 + 1, :]
                            )
                            for bh in range(lo_h, h + 1):
                                bslot = bh % REC_BATCH
                                scr = drp.tile([QCW], F32, name="scr", tag="scr")
                                nc.sync.dma_start(out=scr, in_=rinv[bslot * 32:bslot * 32 + 1, :])
                                rbs = recp.tile([HD, QCW], F32, name="rbs", tag="rbs")
                                nc.sync.dma_start(
                                    out=rbs,
                                    in_=bass.AP(tensor=scr.tensor, offset=scr.offset, ap=[[0, HD]] + list(scr.ap)),
                                )
                                nc.vector.tensor_mul(
                                    attnT[bh // 2][(bh % 2) * HD:(bh % 2) * HD + HD, qc * QCW:(qc + 1) * QCW],
                                    pavs[bh][0:HD, :],
                                    rbs,
                                )
                            if h != HL - 1:
                                stag = recp.tile([128, QCW], F32, name="stag", tag="stag")
                                nc.gpsimd.memset(stag, 1.0)
                                rinv = recp.tile([128, QCW], F32, name="rinv", tag="rinv")
                    # ---- projection for this q-chunk's t-tiles ----
                    for tv in range(TPC):
                        tt = qc * TPC + tv
                        yt = ysb.tile([128, D], F32, name="yt", tag="yt")
                        for nch in range(NPCH):
                            py = psav.tile([128, PCH], F32, name="py", tag="pav")
                            for cc in range(NCT):
                                nc.tensor.matmul(
                                    py,
                                    lhsT=attnT[cc][:, tt * 128:(tt + 1) * 128],
                                    rhs=wps[cc][:, nch * PCH:(nch + 1) * PCH],
                                    start=(cc == 0),
                                    stop=(cc == NCT - 1),
                                )
                            nc.vector.tensor_add(
                                yt[:, nch * PCH:(nch + 1) * PCH], py, bpb[:, nch * PCH:(nch + 1) * PCH]
                            )
                        nc.sync.dma_start(out=y[tt * 128:(tt + 1) * 128, :], in_=yt)
    nc.compile()
    return nc


def classify_mask(mask_bool, T):
    """Classify S^T blocks [k-tile 128, q-chunk 512] as skip / full / mixed.

    mask_bool: [T, T] bool, mask_bool[q, k] = attend(q -> k).
    Returns (blocks, patterns): blocks[qc] = list of (ki, pat_idx|None),
    patterns = np.ndarray [n_pat, 128, QCW] float32.
    """
    QCW = min(512, T)
    NQC = T // QCW
    NKT = T // 128
    maskT = mask_bool.T  # [k, q]
    patterns = []
    pat_index = {}
    blocks = []
    for qc in range(NQC):
        row = []
        for ki in range(NKT):
            blk = maskT[ki * 128:(ki + 1) * 128, qc * QCW:(qc + 1) * QCW]
            if not blk.any():
                continue
            if blk.all():
                row.append((ki, None))
                continue
            # tril-offset block? keep iff k <= q, i.e. p <= base + f
            base = qc * QCW - ki * 128
            p = np.arange(128)[:, None]
            f = np.arange(QCW)[None, :]
            if np.array_equal(blk, p <= base + f):
                row.append((ki, ("tri", base)))
                continue
            key = blk.tobytes()
            if key not in pat_index:
                pat_index[key] = len(patterns)
                patterns.append(blk.astype(np.float32))
            row.append((ki, ("pat", pat_index[key])))
        blocks.append(row)
    n_pat = len(patterns)
    if patterns:
        pats = np.stack(patterns)
    else:
        pats = np.zeros((1, 128, QCW), np.float32)
    return blocks, pats, n_pat


_prog_cache = {}


def _get_program(T, D, HL, mask_bool):
    key = (T, D, HL, mask_bool.tobytes())
    if key not in _prog_cache:
        blocks, pats, n_pat = classify_mask(mask_bool, T)
        nc = build_program(T, D, HL, n_pat, blocks)
        _prog_cache[key] = (nc, blocks, pats)
    return _prog_cache[key]


def kernel(x, W_qkv, b_qkv, W_proj, b_proj, mask):
    out, _ = run_attention(x, W_qkv, b_qkv, W_proj, b_proj, mask)
    return out


def run_attention(x, W_qkv, b_qkv, W_proj, b_proj, mask, trace=False):
    x = np.ascontiguousarray(np.asarray(x, dtype=np.float32))
    W_qkv = np.asarray(W_qkv, dtype=np.float32)
    b_qkv = np.asarray(b_qkv, dtype=np.float32)
    W_proj = np.asarray(W_proj, dtype=np.float32)
    b_proj = np.asarray(b_proj, dtype=np.float32)
    Bc, T, D = x.shape
    NH = NH_FULL
    HL = NH // 2  # heads per core (two head-groups)
    CL = HL * HD

    mask_bool = np.asarray(mask)[0, 0] != 0

    nc, blocks, pats = _get_program(T, D, HL, mask_bool)

    in_maps = []
    n_cores = 2 * Bc
    for c in range(n_cores):
        b, g = c // 2, c % 2
        sl = slice(g * CL, (g + 1) * CL)
        in_maps.append({
            "x": np.ascontiguousarray(x[b]).astype(np.float16),
            "wq": np.ascontiguousarray(W_qkv[:, 0 * D:1 * D][:, sl]).astype(np.float16),
            "wk": np.ascontiguousarray(W_qkv[:, 1 * D:2 * D][:, sl]).astype(np.float16),
            "wv": np.ascontiguousarray(W_qkv[:, 2 * D:3 * D][:, sl]).astype(np.float16),
            "bq": np.ascontiguousarray(b_qkv[0 * D:1 * D][sl]),
            "bk": np.ascontiguousarray(b_qkv[1 * D:2 * D][sl]),
            "bv": np.ascontiguousarray(b_qkv[2 * D:3 * D][sl]),
            "wp": np.ascontiguousarray(W_proj[sl, :]).astype(np.float16),
            "bp": b_proj if g == 0 else np.zeros_like(b_proj),
            "mp": pats.astype(np.float16),
        })

    res = run_bass_kernel_spmd(nc, in_maps, list(range(n_cores)), trace=trace)
    out = np.empty((Bc, T, D), np.float32)
    for b in range(Bc):
        out[b] = res.results[2 * b]["y"] + res.results[2 * b + 1]["y"]
    return out, res
